# revision 10
# baseline (speedup 1.0000x reference)
"""Trainium2 Bass kernel for the LogicLayer (difflogic) problem.

out[i, o] = c0[o] + ca[o]*a + cb[o]*b + cab[o]*a*b
  with a = x[i, idx_a[o]], b = x[i, idx_b[o]],
  [c0, ca, cb, cab] = softmax(weights[o]) @ GATE_COEFFS.

Strategy: OUTPUT-sharded across 8 cores (1024 outputs/core, all 4096
batch rows). x transposed on host to xt[8192 feat, 4096 rows] fp16 in
HBM (replicated). Rationale: SWDGE descriptor generation costs ~9ns per
gather index on the Q7s, so gather few (2048/rep) large (8 KiB) rows
rather than many small ones; the 16 SDMA engines then stream at full
bandwidth.

Per rep and core:
  - 8x dma_gather (256 idx each) pull xt[idx_a[o]] / xt[idx_b[o]] rows
    into SBUF tiles [128 outs, 4096 rows] fp16.
  - per 128-output block (coefficients are per-partition scalars):
      t = a*cab + cb     (DVE tensor_scalar dual-op, 4x mode, 1.3us)
      r = a*ca  + c0     (ACT Identity activation, parallel engine)
      t = t*b; o = t + r (DVE tensor_tensor, 2x mode, 2.3us each)
  - y written [1024 outs, 4096 rows] fp16; host transposes/casts back.
"""

import numpy as np

BATCH, IN_DIM, OUT_DIM = 4096, 8192, 8192
N_CORES = 8
OPC = OUT_DIM // N_CORES  # 1024 outputs per core
RA = BATCH                # all 4096 rows per core
P = 128
NBLK = OPC // P           # 8 output blocks per core
NI = 256                  # indices per dma_gather
NCH = OPC // NI           # 4 gather chunks per operand
BPC = NI // P             # 2 blocks per chunk
ICOLS = NI // 16          # idx columns per chunk (16)

GATE_COEFFS = np.array([
    [0, 0, 0, 0], [0, 0, 0, 1], [0, 1, 0, -1], [0, 1, 0, 0],
    [0, 0, 1, -1], [0, 0, 1, 0], [0, 1, 1, -2], [0, 1, 1, -1],
    [1, -1, -1, 1], [1, -1, -1, 2], [1, 0, -1, 0], [1, 0, -1, 1],
    [1, -1, 0, 0], [1, -1, 0, 1], [1, 0, 0, -1], [1, 0, 0, 0],
], dtype=np.float32)  # [16, 4]

_CACHE = {}


def _build_nc(n_reps=1):
    import concourse.bacc as bacc
    import concourse.mybir as mybir
    from concourse.tile import TileContext

    f32 = mybir.dt.float32
    f16 = mybir.dt.float16
    i16 = mybir.dt.int16
    Alu = mybir.AluOpType
    Act = mybir.ActivationFunctionType

    nc = bacc.Bacc("TRN2", target_bir_lowering=False, debug=False,
                   num_devices=N_CORES)
    xt = nc.dram_tensor("xt", [IN_DIM, RA], f16, kind="ExternalInput").ap()
    idxw = nc.dram_tensor("idxw", [P, 2 * NCH * ICOLS], i16,
                          kind="ExternalInput").ap()
    coef = nc.dram_tensor("coef", [P, 4, NBLK], f32,
                          kind="ExternalInput").ap()
    y = nc.dram_tensor("y", [OPC, RA], f16, kind="ExternalOutput").ap()

    with TileContext(nc) as tc:
        with tc.tile_pool(name="const", bufs=1) as cpool, \
             tc.tile_pool(name="gab", bufs=4) as gpool, \
             tc.tile_pool(name="tr", bufs=2) as tpool, \
             tc.tile_pool(name="out", bufs=2) as opool:
            idx_sb = cpool.tile([P, 2 * NCH * ICOLS], i16, tag="idx")
            nc.sync.dma_start(out=idx_sb[:], in_=idxw)
            cf = cpool.tile([P, 4, NBLK], f32, tag="coef")
            nc.sync.dma_start(out=cf[:], in_=coef)

            for rep in range(n_reps):
                for q in range(NCH):
                    ga = gpool.tile([P, BPC, RA], f16, tag="ga")
                    nc.gpsimd.dma_gather(
                        ga[:], xt, idx_sb[:, q * ICOLS:(q + 1) * ICOLS],
                        NI, NI, RA)
                    gb = gpool.tile([P, BPC, RA], f16, tag="gb")
                    nc.gpsimd.dma_gather(
                        gb[:], xt,
                        idx_sb[:, (NCH + q) * ICOLS:(NCH + q + 1) * ICOLS],
                        NI, NI, RA)
                    for j in range(BPC):
                        m = q * BPC + j
                        a = ga[:, j, :]
                        b = gb[:, j, :]
                        t = tpool.tile([P, RA], f16, tag="t")
                        nc.vector.tensor_scalar(
                            t[:], a, cf[:, 3, m:m + 1], cf[:, 2, m:m + 1],
                            Alu.mult, Alu.add)
                        r = tpool.tile([P, RA], f16, tag="r")
                        nc.scalar.activation(
                            r[:], a, Act.Identity,
                            bias=cf[:, 0, m:m + 1], scale=cf[:, 1, m:m + 1])
                        nc.vector.tensor_mul(t[:], t[:], b)
                        o = opool.tile([P, RA], f16, tag="o")
                        nc.vector.tensor_add(o[:], t[:], r[:])
                        nc.sync.dma_start(
                            out=y[m * P:(m + 1) * P, :], in_=o[:])
    nc.compile()
    return nc


def _wrap_idx(seq):
    # dma_gather index layout: unwrapped[i] = idxs[i % 16, i // 16],
    # tiled to 128 partitions (replicated across the 8 Q7 cores).
    m = seq.reshape(len(seq) // 16, 16).T
    return np.tile(m, (P // 16, 1))


def _prep_host(x, weights, idx_a, idx_b):
    x = np.asarray(x, dtype=np.float32)
    w = np.asarray(weights, dtype=np.float32)
    e = np.exp(w - w.max(axis=1, keepdims=True))
    sm = e / e.sum(axis=1, keepdims=True)
    coeffs = (sm @ GATE_COEFFS).astype(np.float32)          # [8192, 4]

    xt = np.ascontiguousarray(x.T.astype(np.float16))       # [8192, 4096]
    ia = np.asarray(idx_a).astype(np.int16)
    ib = np.asarray(idx_b).astype(np.int16)

    idxws, cfs = [], []
    for c in range(N_CORES):
        lo, hi = c * OPC, (c + 1) * OPC
        cols = [_wrap_idx(ia[lo + q * NI:lo + (q + 1) * NI])
                for q in range(NCH)]
        cols += [_wrap_idx(ib[lo + q * NI:lo + (q + 1) * NI])
                 for q in range(NCH)]
        idxws.append(np.ascontiguousarray(np.concatenate(cols, axis=1)))
        # coef[p, k, m] = coeffs[lo + m*128 + p, k]
        cf = coeffs[lo:hi].reshape(NBLK, P, 4).transpose(1, 2, 0)
        cfs.append(np.ascontiguousarray(cf))
    return xt, idxws, cfs


def _in_maps(x, weights, idx_a, idx_b):
    xt, idxws, cfs = _prep_host(x, weights, idx_a, idx_b)
    return [{"xt": xt, "idxw": idxws[c], "coef": cfs[c]}
            for c in range(N_CORES)]


def kernel(x, weights, idx_a, idx_b):
    from concourse.bass_utils import run_bass_kernel_spmd

    in_maps = _in_maps(x, weights, idx_a, idx_b)
    if "nc" not in _CACHE:
        _CACHE["nc"] = _build_nc()
    nc = _CACHE["nc"]
    res = run_bass_kernel_spmd(nc, in_maps, list(range(N_CORES)))
    out = np.concatenate(
        [res.results[c]["y"].T.astype(np.float32) for c in range(N_CORES)],
        axis=1)
    return out


# revision 11
# speedup vs baseline: 1.0503x; 1.0503x over previous
"""Trainium2 Bass kernel for the LogicLayer (difflogic) problem.

out[i, o] = c0[o] + ca[o]*a + cb[o]*b + cab[o]*a*b
  with a = x[i, idx_a[o]], b = x[i, idx_b[o]],
  [c0, ca, cb, cab] = softmax(weights[o]) @ GATE_COEFFS.

Strategy: OUTPUT-sharded across 8 cores (1024 outputs/core, all 4096
batch rows). x transposed on host to xt[8192 feat, 4096 rows] fp16 in
HBM (replicated). Rationale: SWDGE descriptor generation costs ~9ns per
gather index on the Q7s, so gather few (2048/rep) large (8 KiB) rows
rather than many small ones; the 16 SDMA engines then stream at full
bandwidth.

Per rep and core:
  - 8x dma_gather (256 idx each) pull xt[idx_a[o]] / xt[idx_b[o]] rows
    into SBUF tiles [128 outs, 4096 rows] fp16.
  - per 128-output block (coefficients are per-partition scalars):
      t = a*cab + cb     (DVE tensor_scalar dual-op, 4x mode, 1.3us)
      r = a*ca  + c0     (ACT Identity activation, parallel engine)
      t = t*b; o = t + r (DVE tensor_tensor, 2x mode, 2.3us each)
  - y written [1024 outs, 4096 rows] fp16; host transposes/casts back.
"""

import numpy as np

BATCH, IN_DIM, OUT_DIM = 4096, 8192, 8192
N_CORES = 8
OPC = OUT_DIM // N_CORES  # 1024 outputs per core
RA = BATCH                # all 4096 rows per core
P = 128
NBLK = OPC // P           # 8 output blocks per core
NI = 128                  # indices per dma_gather
NCH = OPC // NI           # 8 gather chunks per operand
BPC = NI // P             # 1 block per chunk
ICOLS = NI // 16          # idx columns per chunk (8)

GATE_COEFFS = np.array([
    [0, 0, 0, 0], [0, 0, 0, 1], [0, 1, 0, -1], [0, 1, 0, 0],
    [0, 0, 1, -1], [0, 0, 1, 0], [0, 1, 1, -2], [0, 1, 1, -1],
    [1, -1, -1, 1], [1, -1, -1, 2], [1, 0, -1, 0], [1, 0, -1, 1],
    [1, -1, 0, 0], [1, -1, 0, 1], [1, 0, 0, -1], [1, 0, 0, 0],
], dtype=np.float32)  # [16, 4]

_CACHE = {}


def _build_nc(n_reps=1):
    import concourse.bacc as bacc
    import concourse.mybir as mybir
    from concourse.tile import TileContext

    f32 = mybir.dt.float32
    f16 = mybir.dt.float16
    i16 = mybir.dt.int16
    Alu = mybir.AluOpType
    Act = mybir.ActivationFunctionType

    nc = bacc.Bacc("TRN2", target_bir_lowering=False, debug=False,
                   num_devices=N_CORES)
    xt = nc.dram_tensor("xt", [IN_DIM, RA], f16, kind="ExternalInput").ap()
    idxw = nc.dram_tensor("idxw", [P, 2 * NCH * ICOLS], i16,
                          kind="ExternalInput").ap()
    coef = nc.dram_tensor("coef", [P, 4, NBLK], f32,
                          kind="ExternalInput").ap()
    y = nc.dram_tensor("y", [OPC, RA], f16, kind="ExternalOutput").ap()

    with TileContext(nc) as tc:
        with tc.tile_pool(name="const", bufs=1) as cpool, \
             tc.tile_pool(name="gab", bufs=9) as gpool, \
             tc.tile_pool(name="tr", bufs=3) as tpool:
            idx_sb = cpool.tile([P, 2 * NCH * ICOLS], i16, tag="idx")
            nc.sync.dma_start(out=idx_sb[:], in_=idxw)
            cf = cpool.tile([P, 4, NBLK], f32, tag="coef")
            nc.sync.dma_start(out=cf[:], in_=coef)

            for rep in range(n_reps):
                for q in range(NCH):
                    ga = gpool.tile([P, BPC, RA], f16, tag="ga")
                    nc.gpsimd.dma_gather(
                        ga[:], xt, idx_sb[:, q * ICOLS:(q + 1) * ICOLS],
                        NI, NI, RA)
                    gb = gpool.tile([P, BPC, RA], f16, tag="gb")
                    nc.gpsimd.dma_gather(
                        gb[:], xt,
                        idx_sb[:, (NCH + q) * ICOLS:(NCH + q + 1) * ICOLS],
                        NI, NI, RA)
                    for j in range(BPC):
                        m = q * BPC + j
                        a = ga[:, j, :]
                        b = gb[:, j, :]
                        t = tpool.tile([P, RA], f16, tag="t")
                        nc.vector.tensor_scalar(
                            t[:], a, cf[:, 3, m:m + 1], cf[:, 2, m:m + 1],
                            Alu.mult, Alu.add)
                        r = tpool.tile([P, RA], f16, tag="r")
                        nc.scalar.activation(
                            r[:], a, Act.Identity,
                            bias=cf[:, 0, m:m + 1], scale=cf[:, 1, m:m + 1])
                        nc.vector.tensor_mul(t[:], t[:], b)
                        nc.vector.tensor_add(t[:], t[:], r[:])
                        nc.sync.dma_start(
                            out=y[m * P:(m + 1) * P, :], in_=t[:])
    nc.compile()
    return nc


def _wrap_idx(seq):
    # dma_gather index layout: unwrapped[i] = idxs[i % 16, i // 16],
    # tiled to 128 partitions (replicated across the 8 Q7 cores).
    m = seq.reshape(len(seq) // 16, 16).T
    return np.tile(m, (P // 16, 1))


def _prep_host(x, weights, idx_a, idx_b):
    x = np.asarray(x, dtype=np.float32)
    w = np.asarray(weights, dtype=np.float32)
    e = np.exp(w - w.max(axis=1, keepdims=True))
    sm = e / e.sum(axis=1, keepdims=True)
    coeffs = (sm @ GATE_COEFFS).astype(np.float32)          # [8192, 4]

    xt = np.ascontiguousarray(x.T.astype(np.float16))       # [8192, 4096]
    ia = np.asarray(idx_a).astype(np.int16)
    ib = np.asarray(idx_b).astype(np.int16)

    idxws, cfs = [], []
    for c in range(N_CORES):
        lo, hi = c * OPC, (c + 1) * OPC
        cols = [_wrap_idx(ia[lo + q * NI:lo + (q + 1) * NI])
                for q in range(NCH)]
        cols += [_wrap_idx(ib[lo + q * NI:lo + (q + 1) * NI])
                 for q in range(NCH)]
        idxws.append(np.ascontiguousarray(np.concatenate(cols, axis=1)))
        # coef[p, k, m] = coeffs[lo + m*128 + p, k]
        cf = coeffs[lo:hi].reshape(NBLK, P, 4).transpose(1, 2, 0)
        cfs.append(np.ascontiguousarray(cf))
    return xt, idxws, cfs


def _in_maps(x, weights, idx_a, idx_b):
    xt, idxws, cfs = _prep_host(x, weights, idx_a, idx_b)
    return [{"xt": xt, "idxw": idxws[c], "coef": cfs[c]}
            for c in range(N_CORES)]


def kernel(x, weights, idx_a, idx_b):
    from concourse.bass_utils import run_bass_kernel_spmd

    in_maps = _in_maps(x, weights, idx_a, idx_b)
    if "nc" not in _CACHE:
        _CACHE["nc"] = _build_nc()
    nc = _CACHE["nc"]
    res = run_bass_kernel_spmd(nc, in_maps, list(range(N_CORES)))
    out = np.concatenate(
        [res.results[c]["y"].T.astype(np.float32) for c in range(N_CORES)],
        axis=1)
    return out


# revision 12
# speedup vs baseline: 1.3048x; 1.2423x over previous
"""Trainium2 Bass kernel for the LogicLayer (difflogic) problem.

out[i, o] = c0[o] + ca[o]*a + cb[o]*b + cab[o]*a*b
  with a = x[i, idx_a[o]], b = x[i, idx_b[o]],
  [c0, ca, cb, cab] = softmax(weights[o]) @ GATE_COEFFS.

Strategy: OUTPUT-sharded across 8 cores (1024 outputs/core, all 4096
batch rows). x transposed on host to xt[8192 feat, 4096 rows] fp16 in
HBM (replicated). Rationale: SWDGE descriptor generation costs ~9ns per
gather index on the Q7s, so gather few (2048/rep) large (8 KiB) rows
rather than many small ones; the 16 SDMA engines then stream at full
bandwidth.

Per rep and core:
  - 8x dma_gather (256 idx each) pull xt[idx_a[o]] / xt[idx_b[o]] rows
    into SBUF tiles [128 outs, 4096 rows] fp16.
  - per 128-output block (coefficients are per-partition scalars):
      t = a*cab + cb     (DVE tensor_scalar dual-op, 4x mode, 1.3us)
      r = a*ca  + c0     (ACT Identity activation, parallel engine)
      t = t*b; o = t + r (DVE tensor_tensor, 2x mode, 2.3us each)
  - y written [1024 outs, 4096 rows] fp16; host transposes/casts back.
"""

import numpy as np

BATCH, IN_DIM, OUT_DIM = 4096, 8192, 8192
N_CORES = 8
OPC = OUT_DIM // N_CORES  # 1024 outputs per core
RA = BATCH                # all 4096 rows per core
P = 128
NBLK = OPC // P           # 8 output blocks per core
NI = 128                  # indices per dma_gather
NCH = OPC // NI           # 8 gather chunks per operand
BPC = NI // P             # 1 block per chunk
ICOLS = NI // 16          # idx columns per chunk (8)

GATE_COEFFS = np.array([
    [0, 0, 0, 0], [0, 0, 0, 1], [0, 1, 0, -1], [0, 1, 0, 0],
    [0, 0, 1, -1], [0, 0, 1, 0], [0, 1, 1, -2], [0, 1, 1, -1],
    [1, -1, -1, 1], [1, -1, -1, 2], [1, 0, -1, 0], [1, 0, -1, 1],
    [1, -1, 0, 0], [1, -1, 0, 1], [1, 0, 0, -1], [1, 0, 0, 0],
], dtype=np.float32)  # [16, 4]

_CACHE = {}


def _build_nc(n_reps=1):
    import concourse.bacc as bacc
    import concourse.mybir as mybir
    from concourse.tile import TileContext

    f32 = mybir.dt.float32
    f16 = mybir.dt.float16
    i16 = mybir.dt.int16
    Alu = mybir.AluOpType
    Act = mybir.ActivationFunctionType

    nc = bacc.Bacc("TRN2", target_bir_lowering=False, debug=False,
                   num_devices=N_CORES, num_swdge_queues=4)
    xt = nc.dram_tensor("xt", [IN_DIM, RA], f16, kind="ExternalInput").ap()
    idxw = nc.dram_tensor("idxw", [P, 2 * NCH * ICOLS], i16,
                          kind="ExternalInput").ap()
    coef = nc.dram_tensor("coef", [P, 4, NBLK], f32,
                          kind="ExternalInput").ap()
    y = nc.dram_tensor("y", [OPC, RA], f16, kind="ExternalOutput").ap()

    with TileContext(nc) as tc:
        with tc.tile_pool(name="const", bufs=1) as cpool, \
             tc.tile_pool(name="gab", bufs=9) as gpool, \
             tc.tile_pool(name="tr", bufs=3) as tpool:
            idx_sb = cpool.tile([P, 2 * NCH * ICOLS], i16, tag="idx")
            nc.sync.dma_start(out=idx_sb[:], in_=idxw)
            cf = cpool.tile([P, 4, NBLK], f32, tag="coef")
            nc.sync.dma_start(out=cf[:], in_=coef)

            for rep in range(n_reps):
                for q in range(NCH):
                    ga = gpool.tile([P, BPC, RA], f16, tag="ga")
                    nc.gpsimd.dma_gather(
                        ga[:], xt, idx_sb[:, q * ICOLS:(q + 1) * ICOLS],
                        NI, NI, RA, queue_num=(2 * q) % 4)
                    gb = gpool.tile([P, BPC, RA], f16, tag="gb")
                    nc.gpsimd.dma_gather(
                        gb[:], xt,
                        idx_sb[:, (NCH + q) * ICOLS:(NCH + q + 1) * ICOLS],
                        NI, NI, RA, queue_num=(2 * q + 1) % 4)
                    for j in range(BPC):
                        m = q * BPC + j
                        a = ga[:, j, :]
                        b = gb[:, j, :]
                        t = tpool.tile([P, RA], f16, tag="t")
                        nc.vector.tensor_scalar(
                            t[:], a, cf[:, 3, m:m + 1], cf[:, 2, m:m + 1],
                            Alu.mult, Alu.add)
                        r = tpool.tile([P, RA], f16, tag="r")
                        nc.scalar.activation(
                            r[:], a, Act.Identity,
                            bias=cf[:, 0, m:m + 1], scale=cf[:, 1, m:m + 1])
                        nc.vector.tensor_mul(t[:], t[:], b)
                        nc.vector.tensor_add(t[:], t[:], r[:])
                        nc.sync.dma_start(
                            out=y[m * P:(m + 1) * P, :], in_=t[:])
    nc.compile()
    return nc


def _wrap_idx(seq):
    # dma_gather index layout: unwrapped[i] = idxs[i % 16, i // 16],
    # tiled to 128 partitions (replicated across the 8 Q7 cores).
    m = seq.reshape(len(seq) // 16, 16).T
    return np.tile(m, (P // 16, 1))


def _prep_host(x, weights, idx_a, idx_b):
    x = np.asarray(x, dtype=np.float32)
    w = np.asarray(weights, dtype=np.float32)
    e = np.exp(w - w.max(axis=1, keepdims=True))
    sm = e / e.sum(axis=1, keepdims=True)
    coeffs = (sm @ GATE_COEFFS).astype(np.float32)          # [8192, 4]

    xt = np.ascontiguousarray(x.T.astype(np.float16))       # [8192, 4096]
    ia = np.asarray(idx_a).astype(np.int16)
    ib = np.asarray(idx_b).astype(np.int16)

    idxws, cfs = [], []
    for c in range(N_CORES):
        lo, hi = c * OPC, (c + 1) * OPC
        cols = [_wrap_idx(ia[lo + q * NI:lo + (q + 1) * NI])
                for q in range(NCH)]
        cols += [_wrap_idx(ib[lo + q * NI:lo + (q + 1) * NI])
                 for q in range(NCH)]
        idxws.append(np.ascontiguousarray(np.concatenate(cols, axis=1)))
        # coef[p, k, m] = coeffs[lo + m*128 + p, k]
        cf = coeffs[lo:hi].reshape(NBLK, P, 4).transpose(1, 2, 0)
        cfs.append(np.ascontiguousarray(cf))
    return xt, idxws, cfs


def _in_maps(x, weights, idx_a, idx_b):
    xt, idxws, cfs = _prep_host(x, weights, idx_a, idx_b)
    return [{"xt": xt, "idxw": idxws[c], "coef": cfs[c]}
            for c in range(N_CORES)]


def kernel(x, weights, idx_a, idx_b):
    from concourse.bass_utils import run_bass_kernel_spmd

    in_maps = _in_maps(x, weights, idx_a, idx_b)
    if "nc" not in _CACHE:
        _CACHE["nc"] = _build_nc()
    nc = _CACHE["nc"]
    res = run_bass_kernel_spmd(nc, in_maps, list(range(N_CORES)))
    out = np.concatenate(
        [res.results[c]["y"].T.astype(np.float32) for c in range(N_CORES)],
        axis=1)
    return out
